# revision 44
# baseline (speedup 1.0000x reference)
"""BDCovpool + Triuvec kernel for Trainium2 (8 NeuronCores, data-parallel).

Math (per sample b, x[b]: [M=196, D=512], t: scalar):
  gram[i,j] = sum_m x[m,i] x[m,j]           (D x D)
  d[i]      = gram[i,i]
  dpre      = d[i] + d[j] - 2 gram
  dcov      = sqrt(exp(t) * relu(dpre) + 1e-5)
  cent      = dcov - rowmean - colmean + totmean   (dcov symmetric -> row==col)
  out       = upper triangle of cent, row-major (131328 per sample)

Device strategy per core (32 samples, processed in 16 pairs):
  The tensor engine runs an upper-block gram stream with zero cross-engine
  deps. For row-block r the moving rhs is sliced to columns >= 128r (block 1
  keeps full width so the PSUM packs gap-free into 3 banks):
    PSUM[128,1408] = [blk0 512 | blk1 512 | blk2 256 | blk3 128]
                   = gram_blocks - (d'_i + d'_j)/2 - gamma*I_blockdiag
  The affine d' correction rides INSIDE the K=70 matmul via two host-packed
  augmentation rows on asymmetric lhsT/rhs tiles:
    xbl = [x(68) ; ones ; negh],  xbr = [x(68) ; negh ; ones],
  negh = -(d-196)/2 computed on host (0.4% of FLOPs). gamma=256 keeps the
  noisy ~0 diagonal sqrt argument positive.
  dcov = ONE ACT Sqrt(PSUM * (-2 e^t) + (392 e^t + eps)) -> fp16 [128,1408]
  (constant bias since both d' halves are in PSUM; fp16 keeps the ~1.0
  dcov values to ~3.5e-4 abs, vs 2e-3 for bf16 which would fail after
  centering where the signal rms is 0.074).
  Row sums: per-block sums on DVE (tensor_scalar+accum_out, fp16 2x) + the
  missing below-diag parts from 5 tiny transposed-colsum matmuls (lhsT =
  dcov slice, rhs = ones column, ap=1) accumulated in PSUM.
  Each sample's tail (colsums, row sums, output DMA) is deferred by TWO
  samples so no engine ever waits on this sample's ACT.
Outputs: the fp16 dcov blocks (ONE DMA per sample, gpsimd queue) + row-sum
columns (one DMA at the end). Host applies the double centering
cent = dcov - c_i - c_j while unpacking rows, and fixes the gamma diagonal.
"""

import numpy as np

B, M, D = 256, 196, 512
NCORES = 8
S = B // NCORES  # samples per core
NPAIR = S // 2  # 16 sample-pairs per core
P = 128
NCH = D // P  # 4 row chunks
MB = M - P  # 68 rows in second k-chunk
MA = MB + 2  # 70 = augmented k-chunk (x rows + 2 correction rows)
GAMMA = 256.0  # diagonal shift; sqrt argument ~2*gamma*e^t ~ 1.3
EPS = 1e-5
KC = float(M)
# device block layout: block r starts at DCQ[r], covers gram cols >= JB[r]
DCQ = [0, 512, 1024, 1280]
JB = [0, 0, 256, 384]
BW = [512, 512, 256, 128]  # block widths
OW = 1408


def build_nc(n_samples=S, fixup=True):
    import concourse.bass as bass
    import concourse.mybir as mybir
    import concourse.tile as tile

    f32 = mybir.dt.float32
    f16 = mybir.dt.float16
    bf16 = mybir.dt.bfloat16
    AF = mybir.ActivationFunctionType
    ALU = mybir.AluOpType

    npair = n_samples // 2

    nc = bass.Bass(
        "TRN2",
        target_bir_lowering=False,
        debug=False,
        enable_asserts=False,
    )

    # pair-packed inputs: xal[p, :, k*512:(k+1)*512] = x[2p+k, 0:128, :]
    xal_d = nc.dram_tensor("xal", [npair, P, 2 * D], bf16, kind="ExternalInput").ap()
    # xblr = [xbl-pair | xbr-pair] along free dim (one DMA)
    xblr_d = nc.dram_tensor(
        "xblr", [npair, MA, 4 * D], bf16, kind="ExternalInput"
    ).ap()
    t128_d = nc.dram_tensor("t128", [P, 1], f32, kind="ExternalInput").ap()
    idc_d = nc.dram_tensor("idc", [P, 2 * P], bf16, kind="ExternalInput").ap()
    onesh_d = nc.dram_tensor("onesh", [P, 1], f16, kind="ExternalInput").ap()
    rect_d = nc.dram_tensor(
        "rect", [npair, 2, P, OW], f16, kind="ExternalOutput"
    ).ap()
    # raw row sums (column form); the tiny scalar combine happens on host
    ccf_d = nc.dram_tensor("ccf", [P, 8 * npair], f32, kind="ExternalOutput").ap()

    PGW = 3 * D  # pg = 3 PSUM banks (1536 cols; 1408 used)

    state = {}

    def tail(prev):
        """Row sums + rect DMA for a finished sample (dc long ready)."""
        dc, pr, k = prev
        cc0 = 8 * pr + 4 * k
        # below-diagonal rowsum parts for blocks 1..3: transposed colsums
        pcol = state["pc_pool"].tile([P, NCH], f32, tag="pcol")
        for r in range(1, NCH):
            for rp in range(r):
                seg = DCQ[rp] + P * r - JB[rp]
                nc.tensor.matmul(
                    pcol[:, r : r + 1],
                    dc[:, seg : seg + P],
                    state["onesh"][:],
                    start=(rp == 0),
                    stop=(rp == r - 1),
                )
        # per-block upper-rect row sums (block 1: its 384-col rect only)
        ccf = state["ccf"]
        for r in range(NCH):
            q = DCQ[r] + P * r - JB[r] if r == 1 else DCQ[r]
            w = BW[r] - P if r == 1 else BW[r]
            nc.vector.tensor_reduce(
                ccf[:, cc0 + r : cc0 + r + 1],
                dc[:, q : q + w],
                state["AX"],
                ALU.add,
            )
        # add the below-diagonal parts
        nc.vector.tensor_tensor(
            ccf[:, cc0 + 1 : cc0 + 4],
            ccf[:, cc0 + 1 : cc0 + 4],
            pcol[:, 1:4],
            ALU.add,
        )
        # ship the packed blocks (alternate the two DMA queues)
        eng = nc.gpsimd if k == 0 else nc.sync
        eng.dma_start(rect_d[pr, k, :, :], dc[:])

    with tile.TileContext(nc) as tc:
        with (
            tc.tile_pool(name="const", bufs=1) as cpool,
            tc.tile_pool(name="xa", bufs=3) as xa_pool,
            tc.tile_pool(name="xb", bufs=3) as xb_pool,
            tc.tile_pool(name="dcov", bufs=4) as dc_pool,
            tc.tile_pool(name="pg", bufs=2, space="PSUM") as pg_pool,
            tc.tile_pool(name="pcol", bufs=2, space="PSUM") as pc_pool,
        ):
            # ---- prefetch pair 0 before the consts (shortens fill) ----
            xa0 = xa_pool.tile([P, 2 * D], bf16, tag="xa")
            nc.sync.dma_start(xa0[:], xal_d[0, :, :])
            xblr0 = xb_pool.tile([MA, 4 * D], bf16, tag="xblr")
            nc.sync.dma_start(xblr0[:], xblr_d[0, :, :])
            # ---- once-per-kernel setup ----
            idc = cpool.tile([P, 2 * P], bf16, tag="idc")
            nc.sync.dma_start(idc[:], idc_d[:])
            onesh = cpool.tile([P, 1], f16, tag="onesh")
            nc.sync.dma_start(onesh[:], onesh_d[:])
            t128 = cpool.tile([P, 1], f32, tag="t128")
            nc.sync.dma_start(t128[:], t128_d[:])
            et128 = cpool.tile([P, 1], f32, tag="et128")
            nc.scalar.activation(et128[:], t128[:], AF.Exp)
            scb = cpool.tile([P, 2], f32, tag="scb")
            # scb col0 = -2 e^t (sqrt scale); col1 = 392 e^t + eps (bias const)
            nc.vector.tensor_scalar_mul(scb[:, 0:1], et128[:], -2.0)
            nc.vector.tensor_scalar(
                scb[:, 1:2], et128[:], 2.0 * KC, EPS, ALU.mult, ALU.add
            )
            sc_ap = scb[:, 0:1]
            bias_ap = scb[:, 1:2]
            ccf = cpool.tile([P, 8 * npair], f32, tag="ccf")
            state.update(
                ccf=ccf, onesh=onesh, pc_pool=pc_pool,
                AX=mybir.AxisListType.X,
            )

            pending = []  # samples whose tail is deferred (2 deep)
            ccf_shipped = 0
            for pr in range(npair):
                # ship finished ccf quarters early to shorten the drain
                if pr in (5, 9, 13):
                    hi = 8 * (pr - 1)
                    nc.sync.dma_start(
                        ccf_d[:, ccf_shipped:hi], ccf[:, ccf_shipped:hi]
                    )
                    ccf_shipped = hi
                # ---- load x pair (gram inputs only; no other PE deps) ----
                if pr == 0:
                    xa, xblr = xa0, xblr0
                else:
                    xa = xa_pool.tile([P, 2 * D], bf16, tag="xa")
                    nc.sync.dma_start(xa[:], xal_d[pr, :, :])
                    xblr = xb_pool.tile([MA, 4 * D], bf16, tag="xblr")
                    nc.sync.dma_start(xblr[:], xblr_d[pr, :, :])

                for k in range(2):
                    # ---- upper-block gram + fused corrections ----
                    pg = pg_pool.tile([P, PGW], f32, tag="pg")
                    for r in range(NCH):
                        q = DCQ[r]
                        w = BW[r]
                        lsl = slice(k * D + P * r, k * D + P * (r + 1))
                        usl = slice(k * D + JB[r], (k + 1) * D)
                        nc.tensor.matmul(
                            pg[:, q : q + w], xa[:, lsl], xa[:, usl],
                            start=True, stop=False,
                        )
                        nc.tensor.matmul(
                            pg[:, q : q + w],
                            xblr[:, lsl],
                            xblr[:, 2 * D + k * D + JB[r] : 2 * D + (k + 1) * D],
                            start=False, stop=False,
                        )
                        # - gamma I on the diagonal block
                        dq = q + P * r - JB[r]
                        nc.tensor.matmul(
                            pg[:, dq : dq + P],
                            idc[:, 0:P],
                            idc[:, P : 2 * P],
                            start=False,
                            stop=True,
                        )

                    # ---- deferred tail (2 samples back) ----
                    if len(pending) == 2:
                        tail(pending.pop(0))
                    # ---- dcov = sqrt(pg*(-2e^t) + bias) -> fp16 packed.
                    # One ACT per PSUM bank (no extra cost: the hw splits at
                    # bank crossings anyway) so each starts right after its
                    # bank's matmuls instead of after the whole gram. ----
                    dc = dc_pool.tile([P, OW], f16, tag="dcov")
                    for lo, hi in ((0, 512), (512, 1024), (1024, OW)):
                        nc.scalar.activation(
                            dc[:, lo:hi], pg[:, lo:hi], AF.Sqrt,
                            bias=bias_ap, scale=sc_ap,
                        )
                    pending.append((dc, pr, k))
            for prev in pending:
                tail(prev)
            nc.sync.dma_start(
                ccf_d[:, ccf_shipped:], ccf[:, ccf_shipped:]
            )

    # This walrus build accepts at most ONE sync wait per instruction.
    # Tile may attach several; hoist each extra wait onto its own no-op
    # placed just before the instruction (same engine, so ordering holds).
    if fixup:
        import bass_rust as _br

        for f in nc.m.functions:
            for blk in f.blocks:
                out_list = []
                changed = False
                for ins in blk.instructions:
                    si = getattr(ins, "sync_info", None)
                    if (
                        type(ins).__name__ != "InstNoOp"
                        and si is not None
                        and si.on_wait
                        and len(si.on_wait) > 1
                        and getattr(ins, "engine", None) is not None
                    ):
                        for j, w in enumerate(si.on_wait[:-1]):
                            nop = _br.InstNoOp(
                                name=f"I-w{j}-{ins.name}",
                                engine=ins.engine,
                                ins=[],
                                outs=[],
                            )
                            nop.sync_info = mybir.SyncInfo(
                                on_wait=[w], on_update=[]
                            )
                            out_list.append(nop)
                        ins.sync_info = mybir.SyncInfo(
                            on_wait=[si.on_wait[-1]], on_update=list(si.on_update)
                        )
                        changed = True
                    out_list.append(ins)
                if changed:
                    blk.instructions = out_list
    return nc


def make_consts(t_np):
    """Host-side constant tensors + the diagonal offset delta."""
    import ml_dtypes

    bf = ml_dtypes.bfloat16
    et = np.float32(np.exp(np.float32(np.asarray(t_np).reshape(-1)[0])))
    idc = np.zeros((P, 2 * P), dtype=np.float32)
    idc[:, 0:P] = np.eye(P)
    idc[:, P : 2 * P] = -GAMMA * np.eye(P)
    # diagonal argument of sqrt: 2*gamma*e^t + eps  (dpre_ii ~ 0)
    cval = np.float32(2.0 * GAMMA * et + EPS)
    delta = np.float32(np.sqrt(cval) - np.sqrt(np.float32(EPS)))
    return {
        "idc": idc.astype(bf),
        "onesh": np.ones((P, 1), dtype=np.float16),
    }, float(delta)


# triu assembly indices (static)
_TRIU_ROWSTART = np.zeros(D + 1, dtype=np.int64)
for _i in range(D):
    _TRIU_ROWSTART[_i + 1] = _TRIU_ROWSTART[_i] + (D - _i)
TRIU_LEN = int(_TRIU_ROWSTART[D])  # 131328


def pack_x(xc):
    """xc: [n, M, D] f32 -> bf16 pair-packed (xal, xblr)."""
    import ml_dtypes

    bf = ml_dtypes.bfloat16
    n = xc.shape[0]
    d = np.einsum("smd,smd->sd", xc, xc, dtype=np.float32)
    negh = (-0.5 * (d - KC)).astype(np.float32)
    xb16 = xc.astype(bf)

    def pairify(a):  # [n, R, D] -> [n/2, R, 2D]
        return np.ascontiguousarray(
            a.reshape(n // 2, 2, a.shape[1], D).transpose(0, 2, 1, 3)
        ).reshape(n // 2, a.shape[1], 2 * D)

    xal = pairify(xb16[:, 0:P, :])
    aug_l = np.empty((n, 2, D), dtype=bf)
    aug_l[:, 0, :] = 1.0
    aug_l[:, 1, :] = negh.astype(bf)
    aug_r = np.empty((n, 2, D), dtype=bf)
    aug_r[:, 0, :] = negh.astype(bf)
    aug_r[:, 1, :] = 1.0
    xmid = xb16[:, P:M, :]
    xbl = pairify(np.concatenate([xmid, aug_l], axis=1))
    xbr = pairify(np.concatenate([xmid, aug_r], axis=1))
    xblr = np.ascontiguousarray(np.concatenate([xbl, xbr], axis=2))
    return xal, xblr


def assemble(rect, ccf, delta):
    """rect: [npair,2,P,OW] f16, ccf: [P, 8*npair] raw rowsums -> cent."""
    npair = rect.shape[0]
    n = npair * 2
    d4 = rect.astype(np.float32).reshape(n, P, OW)
    # rs[s, 128r+p] = ccf[p, 8*pr+4*k+r]  (raw rowsums, gamma'd diagonal)
    rs = (
        ccf.reshape(P, npair, 2, NCH)
        .transpose(1, 2, 3, 0)
        .reshape(n, D)
        .astype(np.float64)
    )
    rs -= delta  # remove the gamma-shifted diagonal contribution
    tot = rs.sum(axis=1, keepdims=True) / (D * D)
    c = (rs / D - tot / 2).astype(np.float32)
    out = np.empty((n, TRIU_LEN), dtype=np.float32)
    for r in range(NCH):
        for p in range(P):
            i = P * r + p
            s = _TRIU_ROWSTART[i]
            ln = D - i
            q = DCQ[r] + i - JB[r]
            out[:, s : s + ln] = (
                d4[:, p, q : q + ln] - c[:, i : i + 1] - c[:, i:D]
            )
            # fix the gamma-shifted diagonal entry
            out[:, s] -= delta
    return out


def make_in_maps(x, t):
    consts, delta = make_consts(t)
    t128 = np.broadcast_to(
        np.asarray(t, dtype=np.float32).reshape(1, 1), (P, 1)
    ).copy()
    in_maps = []
    for c in range(NCORES):
        xal, xblr = pack_x(np.asarray(x[c * S : (c + 1) * S], dtype=np.float32))
        m = {"xal": xal, "xblr": xblr, "t128": t128}
        m.update(consts)
        in_maps.append(m)
    return in_maps, delta


_CACHE = {}


def kernel(**inputs):
    import concourse.bass_utils as bass_utils

    x = np.ascontiguousarray(inputs["x"], dtype=np.float32)
    t = np.asarray(inputs["t"], dtype=np.float32)
    assert x.shape == (B, M, D)

    if "nc" not in _CACHE:
        _CACHE["nc"] = build_nc(S)
    nc = _CACHE["nc"]

    in_maps, delta = make_in_maps(x, t)

    res = bass_utils.run_bass_kernel_spmd(nc, in_maps, core_ids=list(range(NCORES)))
    full = np.empty((B, TRIU_LEN), dtype=np.float32)
    for c in range(NCORES):
        full[c * S : (c + 1) * S] = assemble(
            res.results[c]["rect"], res.results[c]["ccf"], delta
        )
    return full


# revision 45
# speedup vs baseline: 1.1681x; 1.1681x over previous
"""BDCovpool + Triuvec kernel for Trainium2 (8 NeuronCores, data-parallel).

Math (per sample b, x[b]: [M=196, D=512], t: scalar):
  gram[i,j] = sum_m x[m,i] x[m,j]           (D x D)
  d[i]      = gram[i,i]
  dpre      = d[i] + d[j] - 2 gram
  dcov      = sqrt(exp(t) * relu(dpre) + 1e-5)
  cent      = dcov - rowmean - colmean + totmean   (dcov symmetric -> row==col)
  out       = upper triangle of cent, row-major (131328 per sample)

Device strategy per core (32 samples, processed in 16 pairs):
  The tensor engine runs an upper-block gram stream with zero cross-engine
  deps. For row-block r the moving rhs is sliced to columns >= 128r (block 1
  keeps full width so the PSUM packs gap-free into 3 banks):
    PSUM[128,1408] = [blk0 512 | blk1 512 | blk2 256 | blk3 128]
                   = gram_blocks - (d'_i + d'_j)/2 - gamma*I_blockdiag
  The affine d' correction rides INSIDE the K=70 matmul via two host-packed
  augmentation rows on asymmetric lhsT/rhs tiles:
    xbl = [x(68) ; ones ; negh],  xbr = [x(68) ; negh ; ones],
  negh = -(d-196)/2 computed on host (0.4% of FLOPs). gamma=256 keeps the
  noisy ~0 diagonal sqrt argument positive.
  dcov = ONE ACT Sqrt(PSUM * (-2 e^t) + (392 e^t + eps)) -> fp16 [128,1408]
  (constant bias since both d' halves are in PSUM; fp16 keeps the ~1.0
  dcov values to ~3.5e-4 abs, vs 2e-3 for bf16 which would fail after
  centering where the signal rms is 0.074).
  Row sums: per-block sums on DVE (tensor_scalar+accum_out, fp16 2x) + the
  missing below-diag parts from 5 tiny transposed-colsum matmuls (lhsT =
  dcov slice, rhs = ones column, ap=1) accumulated in PSUM.
  Each sample's tail (colsums, row sums, output DMA) is deferred by TWO
  samples so no engine ever waits on this sample's ACT.
Outputs: the fp16 dcov blocks (ONE DMA per sample, gpsimd queue) + row-sum
columns (one DMA at the end). Host applies the double centering
cent = dcov - c_i - c_j while unpacking rows, and fixes the gamma diagonal.
"""

import numpy as np

B, M, D = 256, 196, 512
NCORES = 8
S = B // NCORES  # samples per core
NPAIR = S // 2  # 16 sample-pairs per core
P = 128
NCH = D // P  # 4 row chunks
MB = M - P  # 68 rows in second k-chunk
MA = MB + 2  # 70 = augmented k-chunk (x rows + 2 correction rows)
GAMMA = 256.0  # diagonal shift; sqrt argument ~2*gamma*e^t ~ 1.3
EPS = 1e-5
KC = float(M)
# device block layout: block r starts at DCQ[r], covers gram cols >= JB[r]
DCQ = [0, 512, 1024, 1280]
JB = [0, 0, 256, 384]
BW = [512, 512, 256, 128]  # block widths
OW = 1408


def build_nc(n_samples=S, fixup=True):
    import concourse.bass as bass
    import concourse.mybir as mybir
    import concourse.tile as tile

    f32 = mybir.dt.float32
    f16 = mybir.dt.float16
    bf16 = mybir.dt.bfloat16
    AF = mybir.ActivationFunctionType
    ALU = mybir.AluOpType

    npair = n_samples // 2

    nc = bass.Bass(
        "TRN2",
        target_bir_lowering=False,
        debug=False,
        enable_asserts=False,
    )

    # pair-packed inputs: xal[p, :, k*512:(k+1)*512] = x[2p+k, 0:128, :]
    xal_d = nc.dram_tensor("xal", [npair, P, 2 * D], bf16, kind="ExternalInput").ap()
    # xblr = [xbl-pair | xbr-pair] along free dim (one DMA)
    xblr_d = nc.dram_tensor(
        "xblr", [npair, MA, 4 * D], bf16, kind="ExternalInput"
    ).ap()
    t128_d = nc.dram_tensor("t128", [P, 1], f32, kind="ExternalInput").ap()
    idc_d = nc.dram_tensor("idc", [P, 2 * P], bf16, kind="ExternalInput").ap()
    onesh_d = nc.dram_tensor("onesh", [P, 1], f16, kind="ExternalInput").ap()
    rect_d = nc.dram_tensor(
        "rect", [npair, 2, P, OW], f16, kind="ExternalOutput"
    ).ap()
    # raw row sums (column form); the tiny scalar combine happens on host
    ccf_d = nc.dram_tensor("ccf", [P, 8 * npair], f32, kind="ExternalOutput").ap()

    PGW = 3 * D  # pg = 3 PSUM banks (1536 cols; 1408 used)

    state = {}

    def tail(prev):
        """Row sums + rect DMA for a finished sample (dc long ready)."""
        dc, pr, k = prev
        cc0 = 8 * pr + 4 * k
        # below-diagonal rowsum parts for blocks 1..3: transposed colsums
        pcol = state["pc_pool"].tile([P, NCH], f32, tag="pcol")
        for r in range(1, NCH):
            for rp in range(r):
                seg = DCQ[rp] + P * r - JB[rp]
                nc.tensor.matmul(
                    pcol[:, r : r + 1],
                    dc[:, seg : seg + P],
                    state["onesh"][:],
                    start=(rp == 0),
                    stop=(rp == r - 1),
                )
        # per-block upper-rect row sums (block 1: its 384-col rect only)
        ccf = state["ccf"]
        for r in range(NCH):
            q = DCQ[r] + P * r - JB[r] if r == 1 else DCQ[r]
            w = BW[r] - P if r == 1 else BW[r]
            nc.vector.tensor_reduce(
                ccf[:, cc0 + r : cc0 + r + 1],
                dc[:, q : q + w],
                state["AX"],
                ALU.add,
            )
        # add the below-diagonal parts
        nc.vector.tensor_tensor(
            ccf[:, cc0 + 1 : cc0 + 4],
            ccf[:, cc0 + 1 : cc0 + 4],
            pcol[:, 1:4],
            ALU.add,
        )
        # ship the packed blocks (alternate the two DMA queues)
        eng = nc.gpsimd if k == 0 else nc.sync
        eng.dma_start(rect_d[pr, k, :, :], dc[:])

    with tile.TileContext(nc) as tc:
        with (
            tc.tile_pool(name="const", bufs=1) as cpool,
            tc.tile_pool(name="xa", bufs=3) as xa_pool,
            tc.tile_pool(name="xb", bufs=3) as xb_pool,
            tc.tile_pool(name="dcov", bufs=4) as dc_pool,
            tc.tile_pool(name="pg", bufs=2, space="PSUM") as pg_pool,
            tc.tile_pool(name="pcol", bufs=2, space="PSUM") as pc_pool,
        ):
            # ---- prefetch pair 0 before the consts (shortens fill) ----
            xa0 = xa_pool.tile([P, 2 * D], bf16, tag="xa")
            nc.sync.dma_start(xa0[:], xal_d[0, :, :])
            xblr0 = xb_pool.tile([MA, 4 * D], bf16, tag="xblr")
            nc.sync.dma_start(xblr0[:], xblr_d[0, :, :])
            # ---- once-per-kernel setup ----
            idc = cpool.tile([P, 2 * P], bf16, tag="idc")
            nc.sync.dma_start(idc[:], idc_d[:])
            onesh = cpool.tile([P, 1], f16, tag="onesh")
            nc.sync.dma_start(onesh[:], onesh_d[:])
            t128 = cpool.tile([P, 1], f32, tag="t128")
            nc.sync.dma_start(t128[:], t128_d[:])
            et128 = cpool.tile([P, 1], f32, tag="et128")
            nc.scalar.activation(et128[:], t128[:], AF.Exp)
            scb = cpool.tile([P, 2], f32, tag="scb")
            # scb col0 = -2 e^t (sqrt scale); col1 = 392 e^t + eps (bias const)
            nc.vector.tensor_scalar_mul(scb[:, 0:1], et128[:], -2.0)
            nc.vector.tensor_scalar(
                scb[:, 1:2], et128[:], 2.0 * KC, EPS, ALU.mult, ALU.add
            )
            sc_ap = scb[:, 0:1]
            bias_ap = scb[:, 1:2]
            ccf = cpool.tile([P, 8 * npair], f32, tag="ccf")
            state.update(
                ccf=ccf, onesh=onesh, pc_pool=pc_pool,
                AX=mybir.AxisListType.X,
            )

            pending = []  # samples whose tail is deferred (2 deep)
            ccf_shipped = 0
            for pr in range(npair):
                # ship finished ccf quarters early to shorten the drain
                if pr in (5, 9, 13):
                    hi = 8 * (pr - 1)
                    nc.sync.dma_start(
                        ccf_d[:, ccf_shipped:hi], ccf[:, ccf_shipped:hi]
                    )
                    ccf_shipped = hi
                # ---- load x pair (gram inputs only; no other PE deps) ----
                if pr == 0:
                    xa, xblr = xa0, xblr0
                else:
                    xa = xa_pool.tile([P, 2 * D], bf16, tag="xa")
                    nc.sync.dma_start(xa[:], xal_d[pr, :, :])
                    xblr = xb_pool.tile([MA, 4 * D], bf16, tag="xblr")
                    nc.sync.dma_start(xblr[:], xblr_d[pr, :, :])

                for k in range(2):
                    # ---- upper-block gram + fused corrections ----
                    pg = pg_pool.tile([P, PGW], f32, tag="pg")
                    for r in range(NCH):
                        q = DCQ[r]
                        w = BW[r]
                        lsl = slice(k * D + P * r, k * D + P * (r + 1))
                        usl = slice(k * D + JB[r], (k + 1) * D)
                        nc.tensor.matmul(
                            pg[:, q : q + w], xa[:, lsl], xa[:, usl],
                            start=True, stop=False,
                        )
                        nc.tensor.matmul(
                            pg[:, q : q + w],
                            xblr[:, lsl],
                            xblr[:, 2 * D + k * D + JB[r] : 2 * D + (k + 1) * D],
                            start=False, stop=False,
                        )
                        # - gamma I on the diagonal block
                        dq = q + P * r - JB[r]
                        nc.tensor.matmul(
                            pg[:, dq : dq + P],
                            idc[:, 0:P],
                            idc[:, P : 2 * P],
                            start=False,
                            stop=True,
                        )

                    # ---- deferred tail (2 samples back) ----
                    if len(pending) == 2:
                        tail(pending.pop(0))
                    # ---- dcov = sqrt(pg*(-2e^t) + bias) -> fp16 packed ----
                    dc = dc_pool.tile([P, OW], f16, tag="dcov")
                    nc.scalar.activation(
                        dc[:], pg[:, 0:OW], AF.Sqrt, bias=bias_ap, scale=sc_ap
                    )
                    pending.append((dc, pr, k))
            for prev in pending:
                tail(prev)
            nc.sync.dma_start(
                ccf_d[:, ccf_shipped:], ccf[:, ccf_shipped:]
            )

    # This walrus build accepts at most ONE sync wait per instruction.
    # Tile may attach several; hoist each extra wait onto its own no-op
    # placed just before the instruction (same engine, so ordering holds).
    if fixup:
        import bass_rust as _br

        for f in nc.m.functions:
            for blk in f.blocks:
                out_list = []
                changed = False
                for ins in blk.instructions:
                    si = getattr(ins, "sync_info", None)
                    if (
                        type(ins).__name__ != "InstNoOp"
                        and si is not None
                        and si.on_wait
                        and len(si.on_wait) > 1
                        and getattr(ins, "engine", None) is not None
                    ):
                        for j, w in enumerate(si.on_wait[:-1]):
                            nop = _br.InstNoOp(
                                name=f"I-w{j}-{ins.name}",
                                engine=ins.engine,
                                ins=[],
                                outs=[],
                            )
                            nop.sync_info = mybir.SyncInfo(
                                on_wait=[w], on_update=[]
                            )
                            out_list.append(nop)
                        ins.sync_info = mybir.SyncInfo(
                            on_wait=[si.on_wait[-1]], on_update=list(si.on_update)
                        )
                        changed = True
                    out_list.append(ins)
                if changed:
                    blk.instructions = out_list
    return nc


def make_consts(t_np):
    """Host-side constant tensors + the diagonal offset delta."""
    import ml_dtypes

    bf = ml_dtypes.bfloat16
    et = np.float32(np.exp(np.float32(np.asarray(t_np).reshape(-1)[0])))
    idc = np.zeros((P, 2 * P), dtype=np.float32)
    idc[:, 0:P] = np.eye(P)
    idc[:, P : 2 * P] = -GAMMA * np.eye(P)
    # diagonal argument of sqrt: 2*gamma*e^t + eps  (dpre_ii ~ 0)
    cval = np.float32(2.0 * GAMMA * et + EPS)
    delta = np.float32(np.sqrt(cval) - np.sqrt(np.float32(EPS)))
    return {
        "idc": idc.astype(bf),
        "onesh": np.ones((P, 1), dtype=np.float16),
    }, float(delta)


# triu assembly indices (static)
_TRIU_ROWSTART = np.zeros(D + 1, dtype=np.int64)
for _i in range(D):
    _TRIU_ROWSTART[_i + 1] = _TRIU_ROWSTART[_i] + (D - _i)
TRIU_LEN = int(_TRIU_ROWSTART[D])  # 131328


def pack_x(xc):
    """xc: [n, M, D] f32 -> bf16 pair-packed (xal, xblr)."""
    import ml_dtypes

    bf = ml_dtypes.bfloat16
    n = xc.shape[0]
    d = np.einsum("smd,smd->sd", xc, xc, dtype=np.float32)
    negh = (-0.5 * (d - KC)).astype(np.float32)
    xb16 = xc.astype(bf)

    def pairify(a):  # [n, R, D] -> [n/2, R, 2D]
        return np.ascontiguousarray(
            a.reshape(n // 2, 2, a.shape[1], D).transpose(0, 2, 1, 3)
        ).reshape(n // 2, a.shape[1], 2 * D)

    xal = pairify(xb16[:, 0:P, :])
    aug_l = np.empty((n, 2, D), dtype=bf)
    aug_l[:, 0, :] = 1.0
    aug_l[:, 1, :] = negh.astype(bf)
    aug_r = np.empty((n, 2, D), dtype=bf)
    aug_r[:, 0, :] = negh.astype(bf)
    aug_r[:, 1, :] = 1.0
    xmid = xb16[:, P:M, :]
    xbl = pairify(np.concatenate([xmid, aug_l], axis=1))
    xbr = pairify(np.concatenate([xmid, aug_r], axis=1))
    xblr = np.ascontiguousarray(np.concatenate([xbl, xbr], axis=2))
    return xal, xblr


def assemble(rect, ccf, delta):
    """rect: [npair,2,P,OW] f16, ccf: [P, 8*npair] raw rowsums -> cent."""
    npair = rect.shape[0]
    n = npair * 2
    d4 = rect.astype(np.float32).reshape(n, P, OW)
    # rs[s, 128r+p] = ccf[p, 8*pr+4*k+r]  (raw rowsums, gamma'd diagonal)
    rs = (
        ccf.reshape(P, npair, 2, NCH)
        .transpose(1, 2, 3, 0)
        .reshape(n, D)
        .astype(np.float64)
    )
    rs -= delta  # remove the gamma-shifted diagonal contribution
    tot = rs.sum(axis=1, keepdims=True) / (D * D)
    c = (rs / D - tot / 2).astype(np.float32)
    out = np.empty((n, TRIU_LEN), dtype=np.float32)
    for r in range(NCH):
        for p in range(P):
            i = P * r + p
            s = _TRIU_ROWSTART[i]
            ln = D - i
            q = DCQ[r] + i - JB[r]
            out[:, s : s + ln] = (
                d4[:, p, q : q + ln] - c[:, i : i + 1] - c[:, i:D]
            )
            # fix the gamma-shifted diagonal entry
            out[:, s] -= delta
    return out


def make_in_maps(x, t):
    consts, delta = make_consts(t)
    t128 = np.broadcast_to(
        np.asarray(t, dtype=np.float32).reshape(1, 1), (P, 1)
    ).copy()
    in_maps = []
    for c in range(NCORES):
        xal, xblr = pack_x(np.asarray(x[c * S : (c + 1) * S], dtype=np.float32))
        m = {"xal": xal, "xblr": xblr, "t128": t128}
        m.update(consts)
        in_maps.append(m)
    return in_maps, delta


_CACHE = {}


def kernel(**inputs):
    import concourse.bass_utils as bass_utils

    x = np.ascontiguousarray(inputs["x"], dtype=np.float32)
    t = np.asarray(inputs["t"], dtype=np.float32)
    assert x.shape == (B, M, D)

    if "nc" not in _CACHE:
        _CACHE["nc"] = build_nc(S)
    nc = _CACHE["nc"]

    in_maps, delta = make_in_maps(x, t)

    res = bass_utils.run_bass_kernel_spmd(nc, in_maps, core_ids=list(range(NCORES)))
    full = np.empty((B, TRIU_LEN), dtype=np.float32)
    for c in range(NCORES):
        full[c * S : (c + 1) * S] = assemble(
            res.results[c]["rect"], res.results[c]["ccf"], delta
        )
    return full
